# revision 28
# baseline (speedup 1.0000x reference)
"""NetVLAD Trainium2 Bass kernel, SPMD over 8 NeuronCores.

Contract: kernel(x, Wc, C) takes the FULL inputs
  x  [16, 56, 56, 512] f32, Wc [512, 32] f32, C [512, 32] f32
and returns the FULL output [16, 16384] f32 (matches reference()).

Sharding: data-parallel over batch — 2 samples per core; Wc/C replicated.

Host prep per core: x is cast to bf16 and shipped in BOTH layouts —
x16 [6272 pix, 512 d] (streamed by mm2) and x16T [512 d, 6272 pix]
(stationary chunks for mm1). Same total HBM bytes as one f32 copy
(12.8MB/core); removes all on-chip transposes and dtype converts, so
the PE runs only 6 matmul pairs per 128-pixel tile.

Per-core per 128-pixel tile (49 tiles; D=512, K=32):
  - mm1: s[n,k] = sum_j x16T[j-chunk].T @ Wc[j-chunk]  (4 bf16 matmuls)
  - softmax over k, batched over tile PAIRS to halve ACT/DVE
    instruction count: ACT Exp on [128,2,32], DVE row-sum + reciprocal,
    2x scale -> a bf16
  - mm2: ax[k,d] += a.T @ x16 and a_sum[k] += a.T @ ones (PSUM f32)
  - epilogue: vT = axT + C^T * a_sum, PE-transpose to [d,k], fused
    intra+global L2 normalization; 1/sqrt(D*ssq) is computed as
    exp(-0.5*ln(D*ssq)) so the whole kernel uses a single ACT table
    (exp+ln) — no mid-kernel table reloads.
x16T is fetched in 512-pixel macro tiles (1KB contiguous per
descriptor); emission is software-pipelined (mm1 t / mm2 t-3, x16
prefetch 6 tiles, x16T prefetch 3 macros) so the in-order PE never
waits on the softmax chain.
Measured end-to-end rel err vs the f32 reference ~2e-3.
"""
import sys

if '/opt/trn_rl_repo' not in sys.path:
    sys.path.insert(0, '/opt/trn_rl_repo')

from contextlib import ExitStack

import numpy as np

N_PIX = 3136
N_SAMP = 2
N_ROWS = N_PIX * N_SAMP
P = 128
NT = N_ROWS // P      # 49
D = 512
K = 32
DC = D // P           # 4
BOUND_T = N_PIX // P  # 24
BOUND_R = N_PIX - BOUND_T * P  # 64
N_CORES = 8
MW = 512              # xT macro-tile width (pixels)
NM = (N_ROWS + MW - 1) // MW  # 13 macro tiles (last is 128 wide)

_cache = {}


def _build():
    import concourse.bacc as bacc
    import concourse.mybir as mybir
    import concourse.tile as tile
    from concourse.bass import ts

    F32 = mybir.dt.float32
    BF16 = mybir.dt.bfloat16
    FP16 = mybir.dt.float16

    nc = bacc.Bacc("TRN2", target_bir_lowering=False, debug=False)

    x16 = nc.declare_dram_parameter("x16", [N_ROWS, D], BF16, isOutput=False)
    x16t = nc.declare_dram_parameter("x16t", [D, N_ROWS], BF16,
                                     isOutput=False)
    wc = nc.declare_dram_parameter("wc16", [D, K], BF16, isOutput=False)
    ct = nc.declare_dram_parameter("ct", [K, D], F32, isOutput=False)
    id32 = nc.declare_dram_parameter("id32", [K, K], F32, isOutput=False)
    ones2 = nc.declare_dram_parameter("ones16", [P, 2], BF16, isOutput=False)
    out = nc.declare_dram_parameter("out", [N_SAMP, DC, P, K], F32,
                                    isOutput=True)
    x16, x16t, wc, ct, out, id32, ones2 = (
        x16.ap(), x16t.ap(), wc.ap(), ct.ap(), out.ap(), id32.ap(),
        ones2.ap())

    with tile.TileContext(nc) as tc, ExitStack() as ctx:
        consts = ctx.enter_context(tc.tile_pool(name="consts", bufs=1))
        xpool = ctx.enter_context(tc.tile_pool(name="xpool", bufs=13))
        xtpool = ctx.enter_context(tc.tile_pool(name="xtpool", bufs=5))
        small = ctx.enter_context(tc.tile_pool(name="small", bufs=5))
        apool = ctx.enter_context(tc.tile_pool(name="apool", bufs=13))
        epil = ctx.enter_context(tc.tile_pool(name="epil", bufs=2))
        stpool = ctx.enter_context(tc.tile_pool(name="stpool", bufs=3))
        spool = ctx.enter_context(tc.tile_pool(name="spool", bufs=3))
        ps_sm = ctx.enter_context(tc.tile_pool(name="ps_sm", bufs=3,
                                               space="PSUM"))
        ps_v = ctx.enter_context(tc.tile_pool(name="ps_v", bufs=1,
                                              space="PSUM"))
        ps_acc = ctx.enter_context(tc.tile_pool(name="ps_acc", bufs=2,
                                                space="PSUM"))

        wc_sb = consts.tile([P, DC, K], BF16)
        nc.sync.dma_start(out=wc_sb, in_=wc.rearrange("(c p) k -> p c k", p=P))
        ct_sb = consts.tile([K, D], F32)
        nc.sync.dma_start(out=ct_sb, in_=ct)
        id32_sb = consts.tile([K, K], F32)
        nc.sync.dma_start(out=id32_sb, in_=id32)
        ones_sb = consts.tile([P, 2], BF16)
        nc.sync.dma_start(out=ones_sb, in_=ones2)

        acc = [ps_acc.tile([K, D], F32, name=f"acc{s}", tag="acc")
               for s in range(N_SAMP)]
        asum_ps = [ps_acc.tile([K, 2], F32, name=f"asumps{s}", tag="asum_ps")
                   for s in range(N_SAMP)]
        started = [False, False]

        xms, xTs, s_pairs, a_pairs = {}, {}, {}, {}
        ep_state = {}

        def load_x16(m):
            w = min(MW, N_ROWS - m * MW) // P
            x_m = xpool.tile([P, w, D], BF16, name="x16_m")
            nc.sync.dma_start(
                out=x_m,
                in_=x16[m * MW:m * MW + w * P, :].rearrange(
                    "(q p) d -> p q d", p=P))
            xms[m] = x_m

        def load_xT(m):
            w = min(MW, N_ROWS - m * MW)
            xT = xtpool.tile([P, DC, w], BF16, name="xT_m")
            nc.sync.dma_start(
                out=xT,
                in_=x16t[:, m * MW:m * MW + w].rearrange(
                    "(c p) n -> p c n", p=P))
            xTs[m] = xT

        def mm1_quad(g):
            # sT[k, pix] = Wc.T @ xT for a whole 512-pixel macro: stationary
            # is the 32-col Wc chunk (cheap LDWEIGHTS), the xT macro streams.
            wpix = min(MW, N_ROWS - g * MW)
            xT = xTs[g]
            sT_ps = ps_sm.tile([K, MW], F32, name="sT_ps", tag="sT")
            for j in range(DC):
                nc.tensor.matmul(sT_ps[:, 0:wpix], wc_sb[:, j, :],
                                 xT[:, j, 0:wpix],
                                 start=(j == 0), stop=(j == DC - 1))
            # cast to fp16 (abs err ~5e-4 on |s|<~8) and XBAR-transpose back
            # to [pix, k] on the DMA engines — no PE LDWEIGHTS involved.
            sT16 = stpool.tile([K, MW], FP16, name="sT16")
            nc.scalar.copy(sT16[:, 0:wpix], sT_ps[:, 0:wpix])
            s16 = spool.tile([P, 4, K], FP16, name="s16")
            # issue from the ACT queue (which just wrote sT16) so the
            # transpose never head-blocks the x16/xT loads on the Sync queue
            nc.scalar.dma_start_transpose(out=s16[:, 0:wpix // P, :],
                                          in_=sT16[:, 0:wpix])
            s_pairs[g] = s16

        def softmax_quad(g):
            w = min(4, NT - 4 * g)
            s16 = s_pairs[g]
            exp_sb = small.tile([P, 4, K], F32, name="exp_sb")
            nc.scalar.activation(exp_sb[:, 0:w, :], s16[:, 0:w, :],
                                 mybir.ActivationFunctionType.Exp)
            sumx = small.tile([P, 4], F32, name="sumx")
            nc.vector.reduce_sum(sumx[:, 0:w], exp_sb[:, 0:w, :],
                                 axis=mybir.AxisListType.X)
            rcp = small.tile([P, 4], F32, name="rcp")
            nc.vector.reciprocal(rcp[:, 0:w], sumx[:, 0:w])
            a16 = apool.tile([P, 4, K], BF16, name="a16")
            for q in range(w):
                nc.vector.tensor_scalar_mul(a16[:, q, :], exp_sb[:, q, :],
                                            rcp[:, q:q + 1])
            a_pairs[g] = a16

        def epilogue_a(s):
            asum_sb = epil.tile([K, 1], F32, name=f"asum{s}", tag="asum")
            nc.vector.tensor_copy(asum_sb, asum_ps[s][:, 0:1])
            vt_sb = epil.tile([K, D], F32, name=f"vt{s}", tag="vt")
            nc.vector.tensor_scalar_mul(vt_sb, ct_sb, asum_sb)
            nc.vector.tensor_add(vt_sb, vt_sb, acc[s][:, :])
            v_ps = ps_v.tile([P, DC, K], F32, name=f"vps{s}", tag="sps")
            for j in range(DC):
                nc.tensor.transpose(v_ps[:, j, :], vt_sb[:, ts(j, P)],
                                    id32_sb)
            v_sb = epil.tile([P, DC, K], F32, name=f"v{s}", tag="v")
            nc.vector.tensor_copy(v_sb, v_ps)
            vsq = epil.tile([P, DC, K], F32, name=f"vsq{s}", tag="vsq")
            nc.vector.tensor_mul(vsq, v_sb, v_sb)
            ssq = epil.tile([P, DC], F32, name=f"ssq{s}", tag="ssq")
            nc.vector.reduce_sum(ssq, vsq, axis=mybir.AxisListType.X)
            ep_state[s] = (v_sb, ssq)

        def epilogue_b(s, late_scale):
            v_sb, ssq = ep_state[s]
            # late_scale is a [P,1] tile holding D whose producer depends on
            # the LAST pair's softmax — pins this Sqrt after all loop Exps so
            # the ACT Exp->Sqrt table reload happens once, at the end, not
            # mid-kernel (the tile scheduler otherwise hoists it).
            snorm = epil.tile([P, DC], F32, name=f"sn{s}", tag="sn")
            nc.scalar.activation(snorm, ssq,
                                 mybir.ActivationFunctionType.Sqrt,
                                 scale=late_scale)
            rmult = epil.tile([P, DC], F32, name=f"rm{s}", tag="rm")
            nc.vector.reciprocal(rmult, snorm)
            for j in range(DC):
                nc.vector.tensor_scalar_mul(v_sb[:, j, :], v_sb[:, j, :],
                                            rmult[:, j:j + 1])
            nc.sync.dma_start(out=out[s].rearrange("c p k -> p c k"),
                              in_=v_sb)

        def mm2_stage(t):
            if t < BOUND_T:
                parts = [(0, 0, P)]
            elif t == BOUND_T:
                parts = [(0, 0, BOUND_R), (1, BOUND_R, P)]
            else:
                parts = [(1, 0, P)]
            a16 = a_pairs[t // 4][:, t % 4, :]
            x_t = xms[t // 4][:, t % 4, :]
            for s, r0, r1 in parts:
                first = not started[s]
                started[s] = True
                last_tile = (t == BOUND_T and s == 0) or \
                            (t == NT - 1 and s == 1)
                nc.tensor.matmul(acc[s][:, :], a16[r0:r1, :],
                                 x_t[r0:r1, :],
                                 start=first, stop=last_tile,
                                 skip_group_check=True)
                nc.tensor.matmul(asum_ps[s][:, :], a16[r0:r1, :],
                                 ones_sb[r0:r1, :],
                                 start=first, stop=last_tile,
                                 skip_group_check=True)
                if last_tile:
                    epilogue_a(s)

        # Two-pass emission. Pass 1: per 512-pixel quad, mm1 (streaming,
        # Wc stationary) + XBAR transpose + softmax, keeping every a16 quad
        # live so mm2 never back-pressures the softmax chain. Pass 2: pure
        # mm2/asum stream — the dynamic list scheduler interleaves these
        # ready matmuls into PE idle gaps, keeping the tensor engine dense.
        for m in range(3):
            load_xT(m)
        for m in range(2):
            load_x16(m)
        for g in range(NM):
            if g + 3 < NM:
                load_xT(g + 3)
            if g + 2 < NM:
                load_x16(g + 2)
            mm1_quad(g)
            softmax_quad(g)
        for t in range(NT):
            mm2_stage(t)
        # [P,1] tile of the constant D, data-dependent on the last pair's
        # softmax so epilogue_b's Sqrt cannot be scheduled mid-kernel.
        late_d = epil.tile([P, 1], F32, name="late_d", tag="late_d")
        nc.vector.tensor_scalar(late_d, a_pairs[(NT - 1) // 4][:, 0, 0:1],
                                0.0, float(D), mybir.AluOpType.mult,
                                mybir.AluOpType.add)
        epilogue_b(0, late_d)
        epilogue_b(1, late_d)

    nc.finalize()
    return nc


def _get_nc():
    if "nc" not in _cache:
        _cache["nc"] = _build()
    return _cache["nc"]


def kernel(x, Wc, C):
    from concourse.bass_utils import run_bass_kernel_spmd

    nc = _get_nc()
    maps = make_inputs(x, Wc, C)
    res = run_bass_kernel_spmd(nc, maps, list(range(N_CORES)))
    outs = [r["out"].reshape(N_SAMP, D * K) for r in res.results]
    return np.concatenate(outs, axis=0)


def make_inputs(x, Wc, C):
    """Host-side prep: shard + cast x to bf16 in both layouts."""
    import ml_dtypes
    bf16 = ml_dtypes.bfloat16

    x = np.asarray(x, dtype=np.float32)
    wc16 = np.ascontiguousarray(
        np.asarray(Wc, dtype=np.float32).astype(bf16))
    ct = np.ascontiguousarray(np.asarray(C, dtype=np.float32).T)
    id32 = np.eye(K, dtype=np.float32)
    ones16 = np.ones((P, 2), dtype=bf16)

    per = x.shape[0] // N_CORES
    maps = []
    for i in range(N_CORES):
        xs = x[i * per:(i + 1) * per].reshape(N_ROWS, D).astype(bf16)
        xs = np.ascontiguousarray(xs)
        xst = np.ascontiguousarray(xs.T)
        maps.append({"x16": xs, "x16t": xst, "wc16": wc16, "ct": ct,
                     "id32": id32, "ones16": ones16})
    return maps


# revision 37
# speedup vs baseline: 1.8617x; 1.8617x over previous
"""NetVLAD Trainium2 Bass kernel, SPMD over 8 NeuronCores.

Contract: kernel(x, Wc, C) takes the FULL inputs
  x  [16, 56, 56, 512] f32, Wc [512, 32] f32, C [512, 32] f32
and returns the FULL output [16, 16384] f32 (matches reference()).

Sharding: data-parallel over batch — 2 samples per core; Wc/C replicated.

Host prep per core: x is cast to bf16 and shipped in BOTH layouts —
x16 [6272 pix, 512 d] (streamed by mm2) and x16T [512 d, 6272 pix]
(stationary chunks for mm1). Same total HBM bytes as one f32 copy
(12.8MB/core); removes all on-chip transposes and dtype converts, so
the PE runs only 6 matmul pairs per 128-pixel tile.

Per-core per 128-pixel tile (49 tiles; D=512, K=32):
  - mm1: s[n,k] = sum_j x16T[j-chunk].T @ Wc[j-chunk]  (4 bf16 matmuls)
  - softmax over k, batched over tile PAIRS to halve ACT/DVE
    instruction count: ACT Exp on [128,2,32], DVE row-sum + reciprocal,
    2x scale -> a bf16
  - mm2: ax[k,d] += a.T @ x16 and a_sum[k] += a.T @ ones (PSUM f32)
  - epilogue: vT = axT + C^T * a_sum, PE-transpose to [d,k], fused
    intra+global L2 normalization; 1/sqrt(D*ssq) is computed as
    exp(-0.5*ln(D*ssq)) so the whole kernel uses a single ACT table
    (exp+ln) — no mid-kernel table reloads.
x16T is fetched in 512-pixel macro tiles (1KB contiguous per
descriptor); emission is software-pipelined (mm1 t / mm2 t-3, x16
prefetch 6 tiles, x16T prefetch 3 macros) so the in-order PE never
waits on the softmax chain.
Measured end-to-end rel err vs the f32 reference ~2e-3.
"""
import sys

if '/opt/trn_rl_repo' not in sys.path:
    sys.path.insert(0, '/opt/trn_rl_repo')

from contextlib import ExitStack

import numpy as np

N_PIX = 3136
N_SAMP = 2
N_ROWS = N_PIX * N_SAMP
P = 128
NT = N_ROWS // P      # 49
D = 512
K = 32
DC = D // P           # 4
BOUND_T = N_PIX // P  # 24
BOUND_R = N_PIX - BOUND_T * P  # 64
N_CORES = 8
MW = 512              # xT macro-tile width (pixels)
NM = (N_ROWS + MW - 1) // MW  # 13 macro tiles (last is 128 wide)

_cache = {}


def _build():
    import concourse.bacc as bacc
    import concourse.mybir as mybir
    import concourse.tile as tile
    from concourse.bass import ts

    F32 = mybir.dt.float32
    BF16 = mybir.dt.bfloat16

    nc = bacc.Bacc("TRN2", target_bir_lowering=False, debug=False)

    x16 = nc.declare_dram_parameter("x16", [N_ROWS, D], BF16, isOutput=False)
    x16t = nc.declare_dram_parameter("x16t", [D, N_ROWS], BF16,
                                     isOutput=False)
    wc = nc.declare_dram_parameter("wc16", [D, K], BF16, isOutput=False)
    ct = nc.declare_dram_parameter("ct", [K, D], F32, isOutput=False)
    id32 = nc.declare_dram_parameter("id32", [K, K], F32, isOutput=False)
    ones2 = nc.declare_dram_parameter("ones16", [P, 2], BF16, isOutput=False)
    out = nc.declare_dram_parameter("out", [N_SAMP, DC, P, K], F32,
                                    isOutput=True)
    x16, x16t, wc, ct, out, id32, ones2 = (
        x16.ap(), x16t.ap(), wc.ap(), ct.ap(), out.ap(), id32.ap(),
        ones2.ap())

    with tile.TileContext(nc) as tc, ExitStack() as ctx:
        consts = ctx.enter_context(tc.tile_pool(name="consts", bufs=1))
        xpool = ctx.enter_context(tc.tile_pool(name="xpool", bufs=5))
        xtpool = ctx.enter_context(tc.tile_pool(name="xtpool", bufs=5))
        small = ctx.enter_context(tc.tile_pool(name="small", bufs=5))
        apool = ctx.enter_context(tc.tile_pool(name="apool", bufs=6))
        epil = ctx.enter_context(tc.tile_pool(name="epil", bufs=2))
        ps_sm = ctx.enter_context(tc.tile_pool(name="ps_sm", bufs=4,
                                               space="PSUM"))
        ps_acc = ctx.enter_context(tc.tile_pool(name="ps_acc", bufs=2,
                                                space="PSUM"))

        wc_sb = consts.tile([P, DC, K], BF16)
        nc.sync.dma_start(out=wc_sb, in_=wc.rearrange("(c p) k -> p c k", p=P))
        ct_sb = consts.tile([K, D], F32)
        nc.sync.dma_start(out=ct_sb, in_=ct)
        id32_sb = consts.tile([K, K], F32)
        nc.sync.dma_start(out=id32_sb, in_=id32)
        ones_sb = consts.tile([P, 2], BF16)
        nc.sync.dma_start(out=ones_sb, in_=ones2)

        acc = [ps_acc.tile([K, D], F32, name=f"acc{s}", tag="acc")
               for s in range(N_SAMP)]
        asum_ps = [ps_acc.tile([K, 2], F32, name=f"asumps{s}", tag="asum_ps")
                   for s in range(N_SAMP)]
        started = [False, False]

        xms, xTs, s_pairs, a_pairs = {}, {}, {}, {}
        ep_state = {}

        def load_x16(m):
            w = min(MW, N_ROWS - m * MW) // P
            x_m = xpool.tile([P, w, D], BF16, name="x16_m")
            nc.sync.dma_start(
                out=x_m,
                in_=x16[m * MW:m * MW + w * P, :].rearrange(
                    "(q p) d -> p q d", p=P))
            xms[m] = x_m

        def load_xT(m):
            w = min(MW, N_ROWS - m * MW)
            xT = xtpool.tile([P, DC, w], BF16, name="xT_m")
            nc.sync.dma_start(
                out=xT,
                in_=x16t[:, m * MW:m * MW + w].rearrange(
                    "(c p) n -> p c n", p=P))
            xTs[m] = xT

        def mm1(t):
            m, off = divmod(t * P, MW)
            xT = xTs[m]
            p, q = divmod(t, 2)
            if q == 0:
                s_pairs[p] = ps_sm.tile([P, 2, K], F32, name="s_ps",
                                        tag="sps")
            s_ps = s_pairs[p]
            for j in range(DC):
                nc.tensor.matmul(s_ps[:, q, :], xT[:, j, off:off + P],
                                 wc_sb[:, j, :],
                                 start=(j == 0), stop=(j == DC - 1))

        def softmax_pair(p):
            w = min(2, NT - 2 * p)
            s_ps = s_pairs[p]
            exp_sb = small.tile([P, 2, K], F32, name="exp_sb")
            nc.scalar.activation(exp_sb[:, 0:w, :], s_ps[:, 0:w, :],
                                 mybir.ActivationFunctionType.Exp)
            sumx = small.tile([P, 2], F32, name="sumx")
            nc.vector.reduce_sum(sumx[:, 0:w], exp_sb[:, 0:w, :],
                                 axis=mybir.AxisListType.X)
            rcp = small.tile([P, 2], F32, name="rcp")
            nc.vector.reciprocal(rcp[:, 0:w], sumx[:, 0:w])
            a16 = apool.tile([P, 2, K], BF16, name="a16")
            for q in range(w):
                nc.vector.tensor_scalar_mul(a16[:, q, :], exp_sb[:, q, :],
                                            rcp[:, q:q + 1])
            a_pairs[p] = a16

        def epilogue_a(s):
            asum_sb = epil.tile([K, 1], F32, name=f"asum{s}", tag="asum")
            nc.vector.tensor_copy(asum_sb, asum_ps[s][:, 0:1])
            vt_sb = epil.tile([K, D], F32, name=f"vt{s}", tag="vt")
            nc.vector.tensor_scalar_mul(vt_sb, ct_sb, asum_sb)
            nc.vector.tensor_add(vt_sb, vt_sb, acc[s][:, :])
            v_ps = ps_sm.tile([P, DC, K], F32, name=f"vps{s}", tag="sps")
            for j in range(DC):
                nc.tensor.transpose(v_ps[:, j, :], vt_sb[:, ts(j, P)],
                                    id32_sb)
            v_sb = epil.tile([P, DC, K], F32, name=f"v{s}", tag="v")
            nc.vector.tensor_copy(v_sb, v_ps)
            vsq = epil.tile([P, DC, K], F32, name=f"vsq{s}", tag="vsq")
            nc.vector.tensor_mul(vsq, v_sb, v_sb)
            ssq = epil.tile([P, DC], F32, name=f"ssq{s}", tag="ssq")
            nc.vector.reduce_sum(ssq, vsq, axis=mybir.AxisListType.X)
            ep_state[s] = (v_sb, ssq)

        def epilogue_b(s, late_scale):
            v_sb, ssq = ep_state[s]
            # late_scale is a [P,1] tile holding D whose producer depends on
            # the LAST pair's softmax — pins this Sqrt after all loop Exps so
            # the ACT Exp->Sqrt table reload happens once, at the end, not
            # mid-kernel (the tile scheduler otherwise hoists it).
            snorm = epil.tile([P, DC], F32, name=f"sn{s}", tag="sn")
            nc.scalar.activation(snorm, ssq,
                                 mybir.ActivationFunctionType.Sqrt,
                                 scale=late_scale)
            rmult = epil.tile([P, DC], F32, name=f"rm{s}", tag="rm")
            nc.vector.reciprocal(rmult, snorm)
            for j in range(DC):
                nc.vector.tensor_scalar_mul(v_sb[:, j, :], v_sb[:, j, :],
                                            rmult[:, j:j + 1])
            nc.sync.dma_start(out=out[s].rearrange("c p k -> p c k"),
                              in_=v_sb)

        def mm2_stage(t):
            if t < BOUND_T:
                parts = [(0, 0, P)]
            elif t == BOUND_T:
                parts = [(0, 0, BOUND_R), (1, BOUND_R, P)]
            else:
                parts = [(1, 0, P)]
            a16 = a_pairs[t // 2][:, t % 2, :]
            x_t = xms[t // 4][:, t % 4, :]
            for s, r0, r1 in parts:
                first = not started[s]
                started[s] = True
                last_tile = (t == BOUND_T and s == 0) or \
                            (t == NT - 1 and s == 1)
                nc.tensor.matmul(acc[s][:, :], a16[r0:r1, :],
                                 x_t[r0:r1, :],
                                 start=first, stop=last_tile,
                                 skip_group_check=True)
                nc.tensor.matmul(asum_ps[s][:, :], a16[r0:r1, :],
                                 ones_sb[r0:r1, :],
                                 start=first, stop=last_tile,
                                 skip_group_check=True)
                if last_tile:
                    epilogue_a(s)

        # prologue: prefetch 3 xT macros and 2 x16 macros
        for m in range(3):
            load_xT(m)
        for m in range(2):
            load_x16(m)

        for i in range(NT + 3):
            if i % 4 == 0:
                if i // 4 + 3 < NM:
                    load_xT(i // 4 + 3)
                if i // 4 + 2 < NM:
                    load_x16(i // 4 + 2)
            if i < NT:
                mm1(i)
                if i % 2 == 1 or i == NT - 1:
                    softmax_pair(i // 2)
            if 0 <= i - 3 < NT:
                mm2_stage(i - 3)
        # [P,1] tile of the constant D, data-dependent on the last pair's
        # softmax so epilogue_b's Sqrt cannot be scheduled mid-kernel.
        late_d = epil.tile([P, 1], F32, name="late_d", tag="late_d")
        nc.vector.tensor_scalar(late_d, a_pairs[(NT - 1) // 2][:, 0, 0:1],
                                0.0, float(D), mybir.AluOpType.mult,
                                mybir.AluOpType.add)
        epilogue_b(0, late_d)
        epilogue_b(1, late_d)

    nc.finalize()
    return nc


def _get_nc():
    if "nc" not in _cache:
        _cache["nc"] = _build()
    return _cache["nc"]


def kernel(x, Wc, C):
    from concourse.bass_utils import run_bass_kernel_spmd

    nc = _get_nc()
    maps = make_inputs(x, Wc, C)
    res = run_bass_kernel_spmd(nc, maps, list(range(N_CORES)))
    outs = [r["out"].reshape(N_SAMP, D * K) for r in res.results]
    return np.concatenate(outs, axis=0)


def make_inputs(x, Wc, C):
    """Host-side prep: shard + cast x to bf16 in both layouts."""
    import ml_dtypes
    bf16 = ml_dtypes.bfloat16

    x = np.asarray(x, dtype=np.float32)
    wc16 = np.ascontiguousarray(
        np.asarray(Wc, dtype=np.float32).astype(bf16))
    ct = np.ascontiguousarray(np.asarray(C, dtype=np.float32).T)
    id32 = np.eye(K, dtype=np.float32)
    ones16 = np.ones((P, 2), dtype=bf16)

    per = x.shape[0] // N_CORES
    maps = []
    for i in range(N_CORES):
        xs = x[i * per:(i + 1) * per].reshape(N_ROWS, D).astype(bf16)
        xs = np.ascontiguousarray(xs)
        xst = np.ascontiguousarray(xs.T)
        maps.append({"x16": xs, "x16t": xst, "wc16": wc16, "ct": ct,
                     "id32": id32, "ones16": ones16})
    return maps


# revision 38
# speedup vs baseline: 1.9327x; 1.0382x over previous
"""NetVLAD Trainium2 Bass kernel, SPMD over 8 NeuronCores.

Contract: kernel(x, Wc, C) takes the FULL inputs
  x  [16, 56, 56, 512] f32, Wc [512, 32] f32, C [512, 32] f32
and returns the FULL output [16, 16384] f32 (matches reference()).

Sharding: data-parallel over batch — 2 samples per core; Wc/C replicated.

Host prep per core: x is cast to bf16 and shipped in BOTH layouts —
x16 [6272 pix, 512 d] (streamed by mm2) and x16T [512 d, 6272 pix]
(stationary chunks for mm1). Same total HBM bytes as one f32 copy
(12.8MB/core); removes all on-chip transposes and dtype converts, so
the PE runs only 6 matmul pairs per 128-pixel tile.

Per-core per 128-pixel tile (49 tiles; D=512, K=32):
  - mm1: s[n,k] = sum_j x16T[j-chunk].T @ Wc[j-chunk]  (4 bf16 matmuls)
  - softmax over k, batched over tile PAIRS to halve ACT/DVE
    instruction count: ACT Exp on [128,2,32], DVE row-sum + reciprocal,
    2x scale -> a bf16
  - mm2: ax[k,d] += a.T @ x16 and a_sum[k] += a.T @ ones (PSUM f32)
  - epilogue: vT = axT + C^T * a_sum, PE-transpose to [d,k], fused
    intra+global L2 normalization; 1/sqrt(D*ssq) is computed as
    exp(-0.5*ln(D*ssq)) so the whole kernel uses a single ACT table
    (exp+ln) — no mid-kernel table reloads.
x16T is fetched in 512-pixel macro tiles (1KB contiguous per
descriptor); emission is software-pipelined (mm1 t / mm2 t-3, x16
prefetch 6 tiles, x16T prefetch 3 macros) so the in-order PE never
waits on the softmax chain.
Measured end-to-end rel err vs the f32 reference ~2e-3.
"""
import sys

if '/opt/trn_rl_repo' not in sys.path:
    sys.path.insert(0, '/opt/trn_rl_repo')

from contextlib import ExitStack

import numpy as np

N_PIX = 3136
N_SAMP = 2
N_ROWS = N_PIX * N_SAMP
P = 128
NT = N_ROWS // P      # 49
D = 512
K = 32
DC = D // P           # 4
BOUND_T = N_PIX // P  # 24
BOUND_R = N_PIX - BOUND_T * P  # 64
N_CORES = 8
MW = 512              # xT macro-tile width (pixels)
NM = (N_ROWS + MW - 1) // MW  # 13 macro tiles (last is 128 wide)

_cache = {}


def _build():
    import concourse.bacc as bacc
    import concourse.mybir as mybir
    import concourse.tile as tile
    from concourse.bass import ts

    F32 = mybir.dt.float32
    BF16 = mybir.dt.bfloat16

    nc = bacc.Bacc("TRN2", target_bir_lowering=False, debug=False)

    x16 = nc.declare_dram_parameter("x16", [N_ROWS, D], BF16, isOutput=False)
    x16t = nc.declare_dram_parameter("x16t", [D, N_ROWS], BF16,
                                     isOutput=False)
    wc = nc.declare_dram_parameter("wc16", [D, K], BF16, isOutput=False)
    ct = nc.declare_dram_parameter("ct", [K, D], F32, isOutput=False)
    id32 = nc.declare_dram_parameter("id32", [K, K], F32, isOutput=False)
    ones2 = nc.declare_dram_parameter("ones16", [P, 2], BF16, isOutput=False)
    out = nc.declare_dram_parameter("out", [N_SAMP, DC, P, K], F32,
                                    isOutput=True)
    x16, x16t, wc, ct, out, id32, ones2 = (
        x16.ap(), x16t.ap(), wc.ap(), ct.ap(), out.ap(), id32.ap(),
        ones2.ap())

    with tile.TileContext(nc) as tc, ExitStack() as ctx:
        consts = ctx.enter_context(tc.tile_pool(name="consts", bufs=1))
        xpool = ctx.enter_context(tc.tile_pool(name="xpool", bufs=5))
        xtpool = ctx.enter_context(tc.tile_pool(name="xtpool", bufs=5))
        small = ctx.enter_context(tc.tile_pool(name="small", bufs=5))
        apool = ctx.enter_context(tc.tile_pool(name="apool", bufs=6))
        epil = ctx.enter_context(tc.tile_pool(name="epil", bufs=2))
        ps_sm = ctx.enter_context(tc.tile_pool(name="ps_sm", bufs=4,
                                               space="PSUM"))
        ps_acc = ctx.enter_context(tc.tile_pool(name="ps_acc", bufs=2,
                                                space="PSUM"))

        wc_sb = consts.tile([P, DC, K], BF16)
        nc.sync.dma_start(out=wc_sb, in_=wc.rearrange("(c p) k -> p c k", p=P))
        ct_sb = consts.tile([K, D], F32)
        nc.sync.dma_start(out=ct_sb, in_=ct)
        id32_sb = consts.tile([K, K], F32)
        nc.sync.dma_start(out=id32_sb, in_=id32)
        ones_sb = consts.tile([P, 2], BF16)
        nc.sync.dma_start(out=ones_sb, in_=ones2)

        acc = [ps_acc.tile([K, D], F32, name=f"acc{s}", tag="acc")
               for s in range(N_SAMP)]
        asum_ps = [ps_acc.tile([K, 2], F32, name=f"asumps{s}", tag="asum_ps")
                   for s in range(N_SAMP)]
        started = [False, False]

        xms, xTs, s_pairs, a_pairs = {}, {}, {}, {}
        ep_state = {}

        def load_x16(m):
            w = min(MW, N_ROWS - m * MW) // P
            x_m = xpool.tile([P, w, D], BF16, name="x16_m")
            nc.sync.dma_start(
                out=x_m,
                in_=x16[m * MW:m * MW + w * P, :].rearrange(
                    "(q p) d -> p q d", p=P))
            xms[m] = x_m

        def load_xT(m):
            w = min(MW, N_ROWS - m * MW)
            xT = xtpool.tile([P, DC, w], BF16, name="xT_m")
            nc.sync.dma_start(
                out=xT,
                in_=x16t[:, m * MW:m * MW + w].rearrange(
                    "(c p) n -> p c n", p=P))
            xTs[m] = xT

        def mm1(t):
            m, off = divmod(t * P, MW)
            xT = xTs[m]
            p, q = divmod(t, 2)
            if q == 0:
                s_pairs[p] = ps_sm.tile([P, 2, K], F32, name="s_ps",
                                        tag="sps")
            s_ps = s_pairs[p]
            for j in range(DC):
                nc.tensor.matmul(s_ps[:, q, :], xT[:, j, off:off + P],
                                 wc_sb[:, j, :],
                                 start=(j == 0), stop=(j == DC - 1))

        def softmax_pair(p):
            w = min(2, NT - 2 * p)
            s_ps = s_pairs[p]
            exp_sb = small.tile([P, 2, K], F32, name="exp_sb")
            nc.scalar.activation(exp_sb[:, 0:w, :], s_ps[:, 0:w, :],
                                 mybir.ActivationFunctionType.Exp)
            sumx = small.tile([P, 2], F32, name="sumx")
            nc.vector.reduce_sum(sumx[:, 0:w], exp_sb[:, 0:w, :],
                                 axis=mybir.AxisListType.X)
            rcp = small.tile([P, 2], F32, name="rcp")
            nc.vector.reciprocal(rcp[:, 0:w], sumx[:, 0:w])
            a16 = apool.tile([P, 2, K], BF16, name="a16")
            # split the two scale muls across ACT and DVE: shortens the
            # softmax-chain tail that gates mm2 in the frozen schedule
            nc.scalar.mul(a16[:, 0, :], exp_sb[:, 0, :], rcp[:, 0:1])
            if w > 1:
                nc.vector.tensor_scalar_mul(a16[:, 1, :], exp_sb[:, 1, :],
                                            rcp[:, 1:2])
            a_pairs[p] = a16

        def epilogue_a(s):
            asum_sb = epil.tile([K, 1], F32, name=f"asum{s}", tag="asum")
            nc.vector.tensor_copy(asum_sb, asum_ps[s][:, 0:1])
            vt_sb = epil.tile([K, D], F32, name=f"vt{s}", tag="vt")
            nc.vector.tensor_scalar_mul(vt_sb, ct_sb, asum_sb)
            nc.vector.tensor_add(vt_sb, vt_sb, acc[s][:, :])
            v_ps = ps_sm.tile([P, DC, K], F32, name=f"vps{s}", tag="sps")
            for j in range(DC):
                nc.tensor.transpose(v_ps[:, j, :], vt_sb[:, ts(j, P)],
                                    id32_sb)
            v_sb = epil.tile([P, DC, K], F32, name=f"v{s}", tag="v")
            nc.vector.tensor_copy(v_sb, v_ps)
            vsq = epil.tile([P, DC, K], F32, name=f"vsq{s}", tag="vsq")
            nc.vector.tensor_mul(vsq, v_sb, v_sb)
            ssq = epil.tile([P, DC], F32, name=f"ssq{s}", tag="ssq")
            nc.vector.reduce_sum(ssq, vsq, axis=mybir.AxisListType.X)
            ep_state[s] = (v_sb, ssq)

        def epilogue_b(s, late_scale):
            v_sb, ssq = ep_state[s]
            # late_scale is a [P,1] tile holding D whose producer depends on
            # the LAST pair's softmax — pins this Sqrt after all loop Exps so
            # the ACT Exp->Sqrt table reload happens once, at the end, not
            # mid-kernel (the tile scheduler otherwise hoists it).
            snorm = epil.tile([P, DC], F32, name=f"sn{s}", tag="sn")
            nc.scalar.activation(snorm, ssq,
                                 mybir.ActivationFunctionType.Sqrt,
                                 scale=late_scale)
            rmult = epil.tile([P, DC], F32, name=f"rm{s}", tag="rm")
            nc.vector.reciprocal(rmult, snorm)
            for j in range(DC):
                nc.vector.tensor_scalar_mul(v_sb[:, j, :], v_sb[:, j, :],
                                            rmult[:, j:j + 1])
            nc.sync.dma_start(out=out[s].rearrange("c p k -> p c k"),
                              in_=v_sb)

        def mm2_stage(t):
            if t < BOUND_T:
                parts = [(0, 0, P)]
            elif t == BOUND_T:
                parts = [(0, 0, BOUND_R), (1, BOUND_R, P)]
            else:
                parts = [(1, 0, P)]
            a16 = a_pairs[t // 2][:, t % 2, :]
            x_t = xms[t // 4][:, t % 4, :]
            for s, r0, r1 in parts:
                first = not started[s]
                started[s] = True
                last_tile = (t == BOUND_T and s == 0) or \
                            (t == NT - 1 and s == 1)
                nc.tensor.matmul(acc[s][:, :], a16[r0:r1, :],
                                 x_t[r0:r1, :],
                                 start=first, stop=last_tile,
                                 skip_group_check=True)
                nc.tensor.matmul(asum_ps[s][:, :], a16[r0:r1, :],
                                 ones_sb[r0:r1, :],
                                 start=first, stop=last_tile,
                                 skip_group_check=True)
                if last_tile:
                    epilogue_a(s)

        # prologue: prefetch 3 xT macros and 2 x16 macros
        for m in range(3):
            load_xT(m)
        for m in range(2):
            load_x16(m)

        for i in range(NT + 3):
            if i % 4 == 0:
                if i // 4 + 3 < NM:
                    load_xT(i // 4 + 3)
                if i // 4 + 2 < NM:
                    load_x16(i // 4 + 2)
            if i < NT:
                mm1(i)
                if i % 2 == 1 or i == NT - 1:
                    softmax_pair(i // 2)
            if 0 <= i - 3 < NT:
                mm2_stage(i - 3)
        # [P,1] tile of the constant D, data-dependent on the last pair's
        # softmax so epilogue_b's Sqrt cannot be scheduled mid-kernel.
        late_d = epil.tile([P, 1], F32, name="late_d", tag="late_d")
        nc.vector.tensor_scalar(late_d, a_pairs[(NT - 1) // 2][:, 0, 0:1],
                                0.0, float(D), mybir.AluOpType.mult,
                                mybir.AluOpType.add)
        epilogue_b(0, late_d)
        epilogue_b(1, late_d)

    nc.finalize()
    return nc


def _get_nc():
    if "nc" not in _cache:
        _cache["nc"] = _build()
    return _cache["nc"]


def kernel(x, Wc, C):
    from concourse.bass_utils import run_bass_kernel_spmd

    nc = _get_nc()
    maps = make_inputs(x, Wc, C)
    res = run_bass_kernel_spmd(nc, maps, list(range(N_CORES)))
    outs = [r["out"].reshape(N_SAMP, D * K) for r in res.results]
    return np.concatenate(outs, axis=0)


def make_inputs(x, Wc, C):
    """Host-side prep: shard + cast x to bf16 in both layouts."""
    import ml_dtypes
    bf16 = ml_dtypes.bfloat16

    x = np.asarray(x, dtype=np.float32)
    wc16 = np.ascontiguousarray(
        np.asarray(Wc, dtype=np.float32).astype(bf16))
    ct = np.ascontiguousarray(np.asarray(C, dtype=np.float32).T)
    id32 = np.eye(K, dtype=np.float32)
    ones16 = np.ones((P, 2), dtype=bf16)

    per = x.shape[0] // N_CORES
    maps = []
    for i in range(N_CORES):
        xs = x[i * per:(i + 1) * per].reshape(N_ROWS, D).astype(bf16)
        xs = np.ascontiguousarray(xs)
        xst = np.ascontiguousarray(xs.T)
        maps.append({"x16": xs, "x16t": xst, "wc16": wc16, "ct": ct,
                     "id32": id32, "ones16": ones16})
    return maps
